# revision 42
# baseline (speedup 1.0000x reference)
"""KMeans predict (argmin_k ||x - c_k||^2) on 8 TRN2 NeuronCores.

Data-parallel: x [131072, 768] sharded along N across 8 cores (16384 rows
each), centroid table [1024, 768] replicated. Per core, per 128-token tile:

  argmin_k ||x - c_k||^2  ==  argmax_k (x.c_k - ||c_k||^2/2)

The x.c matmul runs on the PE's double-pumped fp8 path (DoubleRow: 256-deep
contraction per instruction at 0.5 cycles/row, 2x the f32r MAC rate). Full
fp32 accuracy is not needed for an argmax; a 3-term compensated fp8 product
keeps it to ~300 flipped ids out of 131072 (mean rel err ~6e-3, gate 2e-2):

  x.c ~= xh.ch + xh.cl + xl.ch     xh = fp8(x), xl = fp8(x - xh)
                                   ch = fp8(c), cl = fp8(c - ch)

an 18-instruction contraction of 2304 rows per tile (vs f32r's
equivalent-rate 3072), with c pre-scaled by 32 so PSUM holds 32*x.c.

The argmax is restructured so no engine outruns the PE. A classic DVE
max8 + max_index pair costs 2.44us/tile (neither op has a 2-byte fast
mode, and TensorScalarPtr is illegal on Pool), so instead the cluster
index is packed into the score and max_index is eliminated:

  ACT : u = RNE(ps + M2), M2 = 3*2^22 - the fp32 magic-number round; u
        is an exact integer + M2 (ULP 1, quantum 1/32 of x.c)
  ACT : w[232:1024] = u - M2 (Copy with float bias; exact, same binade)
  Pool: w[0:232] = u - M2 (tensor_sub with a memset M2 tile)
  Pool: v[0:352] = w + T[k],  DVE: v[352:1024] = w + T[k]
        T[k] = round(32*bias_k) + k/1024 (exact: 12 int + 10 frac bits)
  DVE : max8(v) written directly into the staging tile (slot 0 = winner;
        packed value v* = S + k/1024, |v| < 2^14)

The tail PE-transposes the staged winner columns in two halves (only the
second sits in the post-loop drain), ACT multiplies by 1024 into int32
(exact), two contiguous DMAs store [16384] i32, and the host unpacks
ids = out % 1024 after the gather. Per-tile engine budget: PE 1.93us
(bound), ACT ~1.84us, Pool ~1.78us, DVE ~1.72us, DMA ~0.6us.

Host-side layout prep (not on the device clock): fp8 hi/lo splits, the
centroid halves pre-scaled by 32 (exact power-of-2 in fp8), x
pre-transposed into DoubleRow tile layout [dlow, sc, j, n] (pairs of
128-row contraction chunks on the free axis), centroids into
[dlow, sc, j, k], T broadcast to [128, K].
"""

import sys

sys.path.insert(0, "/opt/trn_rl_repo")

import ml_dtypes
import numpy as np

N, D, K = 131072, 768, 1024
NCORES = 8
NSH = N // NCORES  # 16384 tokens per core
T = NSH // 128     # 128 token-tiles per core
SC = 3             # 256-row DoubleRow superchunks over D = 768
KHW = 512          # k half-width (one PSUM bank of fp32)
SUBQ = 232         # columns of the magic-sub done by Pool (rest on ACT)
ADDQ = 352         # columns of the pack-add done by Pool (rest on DVE)

NPF8 = ml_dtypes.float8_e4m3
MAGIC = float(3 * 2**22)  # fp32 add of this forces RNE to ULP 1

_nc_cache = []


def _build():
    from concourse import bacc, tile, mybir, masks

    f32 = mybir.dt.float32
    f8 = mybir.dt.float8e4
    i32 = mybir.dt.int32
    DR = mybir.MatmulPerfMode.DoubleRow

    nc = bacc.Bacc("TRN2", target_bir_lowering=False, debug=False)
    # xh[t, p, sc, j, n] = fp8hi(x)[t*128 + n, 256*sc + 128*j + p]
    xh_d = nc.dram_tensor("xh", [T, 128, SC, 2, 128], f8, kind="ExternalInput").ap()
    xl_d = nc.dram_tensor("xl", [T, 128, SC, 2, 128], f8, kind="ExternalInput").ap()
    # ch[p, sc, j, k] = 32*fp8hi(c)[k, 256*sc + 128*j + p]
    ch_d = nc.dram_tensor("ch", [128, SC, 2, K], f8, kind="ExternalInput").ap()
    cl_d = nc.dram_tensor("cl", [128, SC, 2, K], f8, kind="ExternalInput").ap()
    # tpack[p, k] = round(32*bias_k) + k/1024  (broadcast across partitions)
    tp_d = nc.dram_tensor("tpack", [128, K], f32, kind="ExternalInput").ap()
    out = nc.dram_tensor("out", [NSH], i32, kind="ExternalOutput").ap()

    with tile.TileContext(nc) as tc:
        with tc.tile_pool(name="const", bufs=1) as constp:
            ident = constp.tile([128, 128], f32)
            masks.make_identity(nc, ident[:])
            ch_s = constp.tile([128, SC, 2, K], f8)
            cl_s = constp.tile([128, SC, 2, K], f8)
            tp_s = constp.tile([128, K], f32)
            m2_s = constp.tile([128, SUBQ], f32)
            nc.vector.memset(m2_s[:], MAGIC)
            # ---- main loop over token tiles ----
            with tc.tile_pool(name="xin", bufs=3) as xinp, \
                 tc.tile_pool(name="mainps", bufs=3, space="PSUM") as psp, \
                 tc.tile_pool(name="finps", bufs=1, space="PSUM") as finp, \
                 tc.tile_pool(name="uv", bufs=3) as uvp, \
                 tc.tile_pool(name="idxcol", bufs=1) as idxp:
                # tile 0's x DMAs first, then constants in need-order with
                # few big descriptors spread over both staging queues (the
                # HWDGE feeds descriptors serially at ~630ns each)
                xh_0 = xinp.tile([128, SC, 2, 128], f8, tag="xh")
                nc.sync.dma_start(xh_0[:], xh_d[0])
                xl_0 = xinp.tile([128, SC, 2, 128], f8, tag="xl")
                nc.sync.dma_start(xl_0[:], xl_d[0])
                nc.scalar.dma_start(ch_s[:, 0], ch_d[:, 0])
                nc.gpsimd.dma_start(cl_s[:, 0], cl_d[:, 0])
                nc.scalar.dma_start(ch_s[:, 1], ch_d[:, 1])
                nc.gpsimd.dma_start(cl_s[:, 1], cl_d[:, 1])
                nc.scalar.dma_start(ch_s[:, 2], ch_d[:, 2])
                nc.gpsimd.dma_start(cl_s[:, 2], cl_d[:, 2])
                nc.scalar.dma_start(tp_s[:], tp_d[:])

                pcol8 = idxp.tile([128, T, 8], f32)
                for t in range(T):
                    if t == 0:
                        xh_t, xl_t = xh_0, xl_0
                    else:
                        xh_t = xinp.tile([128, SC, 2, 128], f8, tag="xh")
                        nc.sync.dma_start(xh_t[:], xh_d[t])
                        xl_t = xinp.tile([128, SC, 2, 128], f8, tag="xl")
                        nc.sync.dma_start(xl_t[:], xl_d[t])
                    ps = psp.tile([128, K], f32, tag="scps")
                    w = uvp.tile([128, K], f32, tag="w")
                    v = uvp.tile([128, K], f32, tag="v")
                    for kh in range(2):
                        ksl = slice(kh * KHW, (kh + 1) * KHW)
                        for si, (xs, cs) in enumerate(
                            ((xh_t, ch_s), (xh_t, cl_s), (xl_t, ch_s))
                        ):
                            for sci in range(SC):
                                nc.tensor.matmul(
                                    ps[:, ksl],
                                    xs[:, sci, :, :],
                                    cs[:, sci, :, ksl],
                                    start=(si == 0 and sci == 0),
                                    stop=(si == 2 and sci == SC - 1),
                                    perf_mode=DR,
                                )
                    # round: u = RNE(ps + MAGIC) exact integer (ULP 1);
                    # then w = u - MAGIC exact (same binade), split so every
                    # engine stays under the 18-matmul PE period: ACT takes
                    # u and 3/4 of the sub, Pool the other sub quarter.
                    u = uvp.tile([128, K], f32, tag="u")
                    nc.scalar.activation(
                        u[:], ps[:], mybir.ActivationFunctionType.Copy,
                        bias=MAGIC, scale=1.0,
                    )
                    nc.gpsimd.tensor_sub(w[:, 0:SUBQ], u[:, 0:SUBQ], m2_s[:])
                    nc.scalar.activation(
                        w[:, SUBQ:K], u[:, SUBQ:K],
                        mybir.ActivationFunctionType.Copy,
                        bias=-MAGIC, scale=1.0,
                    )
                    # v = w + (round(32b) + k/1024): exact int.frac packing
                    nc.gpsimd.tensor_add(v[:, 0:ADDQ], w[:, 0:ADDQ], tp_s[:, 0:ADDQ])
                    nc.vector.tensor_add(v[:, ADDQ:K], w[:, ADDQ:K], tp_s[:, ADDQ:K])
                    # packed argmax: winner value carries its index; top-8
                    # written straight into the staging tile (slot 0 used)
                    nc.vector.max(pcol8[:, t], v[:])

                    # transpose [token_in_tile, tile] -> [tile,
                    # token_in_tile] in two halves so only the second half
                    # sits in the post-loop drain; *1024 -> int32 makes the
                    # packed value integral (host unpacks % 1024)
                    if t == T // 2 + 3 or t == T - 1:
                        half = 0 if t < T - 1 else 1
                        hsl = slice(half * (T // 2), (half + 1) * (T // 2))
                        ftps = finp.tile([T // 2, T], f32, tag=f"ftps{half}")
                        nc.tensor.transpose(
                            ftps[:, :], pcol8[:, hsl, 0], ident[:]
                        )
                        oi = uvp.tile([T // 2, T], i32, tag=f"oi{half}")
                        nc.scalar.mul(oi[:, :], ftps[:, :], 1024.0)
                        nc.sync.dma_start(
                            out.rearrange("(t p) -> t p", p=128)[hsl], oi[:, :]
                        )

    nc.compile()
    return nc


def _get_nc():
    if not _nc_cache:
        _nc_cache.append(_build())
    return _nc_cache[0]


def _prep(x, centroids):
    x = np.ascontiguousarray(np.asarray(x), dtype=np.float32)
    c = np.ascontiguousarray(np.asarray(centroids), dtype=np.float32)

    ch8 = c.astype(NPF8)
    cl8 = (c - ch8.astype(np.float32)).astype(NPF8)
    # pre-scale by 32 (exact power-of-2 in fp8) so PSUM holds 32*x.c
    ch8 = (ch8.astype(np.float32) * 32.0).astype(NPF8)
    cl8 = (cl8.astype(np.float32) * 32.0).astype(NPF8)
    # [k, sc, j, p] -> [p, sc, j, k]
    ch_a = np.ascontiguousarray(ch8.reshape(K, SC, 2, 128).transpose(3, 1, 2, 0))
    cl_a = np.ascontiguousarray(cl8.reshape(K, SC, 2, 128).transpose(3, 1, 2, 0))

    csq = (c.astype(np.float64) ** 2).sum(-1)
    b = -0.5 * csq
    b -= b.mean()
    tpack = (np.round(32.0 * b) + np.arange(K) / 1024.0).astype(np.float32)
    tp_a = np.ascontiguousarray(np.broadcast_to(tpack.reshape(1, K), (128, K)))

    in_maps = []
    for i in range(NCORES):
        sh = x[i * NSH:(i + 1) * NSH]
        xh8 = sh.astype(NPF8)
        xl8 = (sh - xh8.astype(np.float32)).astype(NPF8)
        # [t, n, sc, j, p] -> [t, p, sc, j, n]
        xh_a = np.ascontiguousarray(
            xh8.reshape(T, 128, SC, 2, 128).transpose(0, 4, 2, 3, 1)
        )
        xl_a = np.ascontiguousarray(
            xl8.reshape(T, 128, SC, 2, 128).transpose(0, 4, 2, 3, 1)
        )
        in_maps.append(
            {"xh": xh_a, "xl": xl_a, "ch": ch_a, "cl": cl_a, "tpack": tp_a}
        )
    return in_maps


def kernel(x, centroids):
    from concourse import bass_utils

    nc = _get_nc()
    in_maps = _prep(x, centroids)
    res = bass_utils.run_bass_kernel_spmd(nc, in_maps, core_ids=list(range(NCORES)))
    packed = np.concatenate([res.results[i]["out"] for i in range(NCORES)])
    return (packed.astype(np.int64) % 1024).astype(np.int32)


# revision 50
# speedup vs baseline: 1.0032x; 1.0032x over previous
"""KMeans predict (argmin_k ||x - c_k||^2) on 8 TRN2 NeuronCores.

Data-parallel: x [131072, 768] sharded along N across 8 cores (16384 rows
each), centroid table [1024, 768] replicated. Per core, per 128-token tile:

  argmin_k ||x - c_k||^2  ==  argmax_k (x.c_k - ||c_k||^2/2)

The x.c matmul runs on the PE's double-pumped fp8 path (DoubleRow: 256-deep
contraction per instruction at 0.5 cycles/row, 2x the f32r MAC rate). Full
fp32 accuracy is not needed for an argmax; a 3-term compensated fp8 product
keeps it to ~300 flipped ids out of 131072 (mean rel err ~6e-3, gate 2e-2):

  x.c ~= xh.ch + xh.cl + xl.ch     xh = fp8(x), xl = fp8(x - xh)
                                   ch = fp8(c), cl = fp8(c - ch)

an 18-instruction contraction of 2304 rows per tile (vs f32r's
equivalent-rate 3072), with c pre-scaled by 32 so PSUM holds 32*x.c.

The argmax is restructured so no engine outruns the PE. A classic DVE
max8 + max_index pair costs 2.44us/tile (neither op has a 2-byte fast
mode, and TensorScalarPtr is illegal on Pool), so instead the cluster
index is packed into the score and max_index is eliminated:

  ACT : u = RNE(ps + M2), M2 = 3*2^22 - the fp32 magic-number round; u
        is an exact integer + M2 (ULP 1, quantum 1/32 of x.c)
  ACT : w[232:1024] = u - M2 (Copy with float bias; exact, same binade)
  Pool: w[0:232] = u - M2 (tensor_sub with a memset M2 tile)
  Pool: v[0:352] = w + T[k],  DVE: v[352:1024] = w + T[k]
        T[k] = round(32*bias_k) + k/1024 (exact: 12 int + 10 frac bits)
  DVE : max8(v) written directly into the staging tile (slot 0 = winner;
        packed value v* = S + k/1024, |v| < 2^14)

The tail PE-transposes the staged winner columns in two halves (only the
second sits in the post-loop drain), ACT multiplies by 1024 into int32
(exact), two contiguous DMAs store [16384] i32, and the host unpacks
ids = out % 1024 after the gather. Per-tile engine budget: PE 1.93us
(bound), ACT ~1.84us, Pool ~1.78us, DVE ~1.72us, DMA ~0.6us.

Host-side layout prep (not on the device clock): fp8 hi/lo splits, the
centroid halves pre-scaled by 32 (exact power-of-2 in fp8), x
pre-transposed into DoubleRow tile layout [dlow, sc, j, n] (pairs of
128-row contraction chunks on the free axis), centroids into
[dlow, sc, j, k], T broadcast to [128, K].
"""

import sys

sys.path.insert(0, "/opt/trn_rl_repo")

import ml_dtypes
import numpy as np

N, D, K = 131072, 768, 1024
NCORES = 8
NSH = N // NCORES  # 16384 tokens per core
T = NSH // 128     # 128 token-tiles per core
SC = 3             # 256-row DoubleRow superchunks over D = 768
KHW = 512          # k half-width (one PSUM bank of fp32)
SUBQ = 232         # columns of the magic-sub done by Pool (rest on ACT)
ADDQ = 352         # columns of the pack-add done by Pool (rest on DVE)

NPF8 = ml_dtypes.float8_e4m3
MAGIC = float(3 * 2**22)  # fp32 add of this forces RNE to ULP 1

_nc_cache = []


def _build():
    from concourse import bacc, tile, mybir, masks

    f32 = mybir.dt.float32
    f8 = mybir.dt.float8e4
    i32 = mybir.dt.int32
    DR = mybir.MatmulPerfMode.DoubleRow

    nc = bacc.Bacc("TRN2", target_bir_lowering=False, debug=False)
    # xh[t, p, sc, j, n] = fp8hi(x)[t*128 + n, 256*sc + 128*j + p]
    xh_d = nc.dram_tensor("xh", [T, 128, SC, 2, 128], f8, kind="ExternalInput").ap()
    xl_d = nc.dram_tensor("xl", [T, 128, SC, 2, 128], f8, kind="ExternalInput").ap()
    # ch[p, sc, j, k] = 32*fp8hi(c)[k, 256*sc + 128*j + p]
    ch_d = nc.dram_tensor("ch", [128, SC, 2, K], f8, kind="ExternalInput").ap()
    cl_d = nc.dram_tensor("cl", [128, SC, 2, K], f8, kind="ExternalInput").ap()
    # tpack[p, k] = round(32*bias_k) + k/1024  (broadcast across partitions)
    tp_d = nc.dram_tensor("tpack", [128, K], f32, kind="ExternalInput").ap()
    out = nc.dram_tensor("out", [NSH], i32, kind="ExternalOutput").ap()

    with tile.TileContext(nc) as tc:
        with tc.tile_pool(name="const", bufs=1) as constp:
            ident = constp.tile([128, 128], f32)
            masks.make_identity(nc, ident[:])
            ch_s = constp.tile([128, SC, 2, K], f8)
            cl_s = constp.tile([128, SC, 2, K], f8)
            tp_s = constp.tile([128, K], f32)
            m2_s = constp.tile([128, SUBQ], f32)
            nc.vector.memset(m2_s[:], MAGIC)
            # ---- main loop over token tiles ----
            with tc.tile_pool(name="xin", bufs=3) as xinp, \
                 tc.tile_pool(name="mainps", bufs=3, space="PSUM") as psp, \
                 tc.tile_pool(name="finps", bufs=1, space="PSUM") as finp, \
                 tc.tile_pool(name="uv", bufs=3) as uvp, \
                 tc.tile_pool(name="idxcol", bufs=1) as idxp:
                # tile 0's x DMAs first, then constants in need-order with
                # few big descriptors spread over both staging queues (the
                # HWDGE feeds descriptors serially at ~630ns each)
                xh_0 = xinp.tile([128, SC, 2, 128], f8, tag="xh")
                nc.sync.dma_start(xh_0[:], xh_d[0])
                xl_0 = xinp.tile([128, SC, 2, 128], f8, tag="xl")
                nc.sync.dma_start(xl_0[:], xl_d[0])
                nc.scalar.dma_start(ch_s[:, 0], ch_d[:, 0])
                nc.gpsimd.dma_start(cl_s[:, 0], cl_d[:, 0])
                nc.scalar.dma_start(ch_s[:, 1], ch_d[:, 1])
                nc.gpsimd.dma_start(cl_s[:, 1], cl_d[:, 1])
                nc.scalar.dma_start(ch_s[:, 2], ch_d[:, 2])
                nc.gpsimd.dma_start(cl_s[:, 2], cl_d[:, 2])
                nc.scalar.dma_start(tp_s[:], tp_d[:])

                pcol8 = idxp.tile([128, T, 8], f32)
                for t in range(T):
                    if t == 0:
                        xh_t, xl_t = xh_0, xl_0
                    else:
                        xh_t = xinp.tile([128, SC, 2, 128], f8, tag="xh")
                        nc.sync.dma_start(xh_t[:], xh_d[t])
                        xl_t = xinp.tile([128, SC, 2, 128], f8, tag="xl")
                        nc.sync.dma_start(xl_t[:], xl_d[t])
                    ps = psp.tile([128, K], f32, tag="scps")
                    w = uvp.tile([128, K], f32, tag="w")
                    v = uvp.tile([128, K], f32, tag="v")
                    # banks interleaved per chunk and terms ordered by
                    # staging-arrival so each landing table chunk feeds 2-4
                    # matmuls, minimizing PE starvation during startup
                    # (accumulation groups are address-disjoint per bank)
                    terms = [
                        (xh_t, ch_s, 0), (xh_t, cl_s, 0), (xl_t, ch_s, 0),
                        (xh_t, ch_s, 1), (xl_t, ch_s, 1), (xh_t, cl_s, 1),
                        (xh_t, ch_s, 2), (xl_t, ch_s, 2), (xh_t, cl_s, 2),
                    ]
                    for ti, (xs, cs, sci) in enumerate(terms):
                        for kh in range(2):
                            ksl = slice(kh * KHW, (kh + 1) * KHW)
                            nc.tensor.matmul(
                                ps[:, ksl],
                                xs[:, sci, :, :],
                                cs[:, sci, :, ksl],
                                start=(ti == 0),
                                stop=(ti == len(terms) - 1),
                                perf_mode=DR,
                            )
                    # round: u = RNE(ps + MAGIC) exact integer (ULP 1);
                    # then w = u - MAGIC exact (same binade), split so every
                    # engine stays under the 18-matmul PE period: ACT takes
                    # u and 3/4 of the sub, Pool the other sub quarter.
                    u = uvp.tile([128, K], f32, tag="u")
                    nc.scalar.activation(
                        u[:], ps[:], mybir.ActivationFunctionType.Copy,
                        bias=MAGIC, scale=1.0,
                    )
                    nc.gpsimd.tensor_sub(w[:, 0:SUBQ], u[:, 0:SUBQ], m2_s[:])
                    nc.scalar.activation(
                        w[:, SUBQ:K], u[:, SUBQ:K],
                        mybir.ActivationFunctionType.Copy,
                        bias=-MAGIC, scale=1.0,
                    )
                    # v = w + (round(32b) + k/1024): exact int.frac packing
                    nc.gpsimd.tensor_add(v[:, 0:ADDQ], w[:, 0:ADDQ], tp_s[:, 0:ADDQ])
                    nc.vector.tensor_add(v[:, ADDQ:K], w[:, ADDQ:K], tp_s[:, ADDQ:K])
                    # packed argmax: winner value carries its index; top-8
                    # written straight into the staging tile (slot 0 used)
                    nc.vector.max(pcol8[:, t], v[:])

                    # transpose [token_in_tile, tile] -> [tile,
                    # token_in_tile] in two halves so only the second half
                    # sits in the post-loop drain; *1024 -> int32 makes the
                    # packed value integral (host unpacks % 1024)
                    if t == T // 2 + 3 or t == T - 1:
                        half = 0 if t < T - 1 else 1
                        hsl = slice(half * (T // 2), (half + 1) * (T // 2))
                        ftps = finp.tile([T // 2, T], f32, tag=f"ftps{half}")
                        nc.tensor.transpose(
                            ftps[:, :], pcol8[:, hsl, 0], ident[:]
                        )
                        oi = uvp.tile([T // 2, T], i32, tag=f"oi{half}")
                        nc.scalar.mul(oi[:, :], ftps[:, :], 1024.0)
                        nc.sync.dma_start(
                            out.rearrange("(t p) -> t p", p=128)[hsl], oi[:, :]
                        )

    nc.compile()
    return nc


def _get_nc():
    if not _nc_cache:
        _nc_cache.append(_build())
    return _nc_cache[0]


def _prep(x, centroids):
    x = np.ascontiguousarray(np.asarray(x), dtype=np.float32)
    c = np.ascontiguousarray(np.asarray(centroids), dtype=np.float32)

    ch8 = c.astype(NPF8)
    cl8 = (c - ch8.astype(np.float32)).astype(NPF8)
    # pre-scale by 32 (exact power-of-2 in fp8) so PSUM holds 32*x.c
    ch8 = (ch8.astype(np.float32) * 32.0).astype(NPF8)
    cl8 = (cl8.astype(np.float32) * 32.0).astype(NPF8)
    # [k, sc, j, p] -> [p, sc, j, k]
    ch_a = np.ascontiguousarray(ch8.reshape(K, SC, 2, 128).transpose(3, 1, 2, 0))
    cl_a = np.ascontiguousarray(cl8.reshape(K, SC, 2, 128).transpose(3, 1, 2, 0))

    csq = (c.astype(np.float64) ** 2).sum(-1)
    b = -0.5 * csq
    b -= b.mean()
    tpack = (np.round(32.0 * b) + np.arange(K) / 1024.0).astype(np.float32)
    tp_a = np.ascontiguousarray(np.broadcast_to(tpack.reshape(1, K), (128, K)))

    in_maps = []
    for i in range(NCORES):
        sh = x[i * NSH:(i + 1) * NSH]
        xh8 = sh.astype(NPF8)
        xl8 = (sh - xh8.astype(np.float32)).astype(NPF8)
        # [t, n, sc, j, p] -> [t, p, sc, j, n]
        xh_a = np.ascontiguousarray(
            xh8.reshape(T, 128, SC, 2, 128).transpose(0, 4, 2, 3, 1)
        )
        xl_a = np.ascontiguousarray(
            xl8.reshape(T, 128, SC, 2, 128).transpose(0, 4, 2, 3, 1)
        )
        in_maps.append(
            {"xh": xh_a, "xl": xl_a, "ch": ch_a, "cl": cl_a, "tpack": tp_a}
        )
    return in_maps


def kernel(x, centroids):
    from concourse import bass_utils

    nc = _get_nc()
    in_maps = _prep(x, centroids)
    res = bass_utils.run_bass_kernel_spmd(nc, in_maps, core_ids=list(range(NCORES)))
    packed = np.concatenate([res.results[i]["out"] for i in range(NCORES)])
    return (packed.astype(np.int64) % 1024).astype(np.int32)


# revision 55
# speedup vs baseline: 1.0040x; 1.0008x over previous
"""KMeans predict (argmin_k ||x - c_k||^2) on 8 TRN2 NeuronCores.

Data-parallel: x [131072, 768] sharded along N across 8 cores (16384 rows
each), centroid table [1024, 768] replicated. Per core, per 128-token tile:

  argmin_k ||x - c_k||^2  ==  argmax_k (x.c_k - ||c_k||^2/2)

The x.c matmul runs on the PE's double-pumped fp8 path (DoubleRow: 256-deep
contraction per instruction at 0.5 cycles/row, 2x the f32r MAC rate). Full
fp32 accuracy is not needed for an argmax; a 3-term compensated fp8 product
keeps it to ~300 flipped ids out of 131072 (mean rel err ~6e-3, gate 2e-2):

  x.c ~= xh.ch + xh.cl + xl.ch     xh = fp8(x), xl = fp8(x - xh)
                                   ch = fp8(c), cl = fp8(c - ch)

an 18-instruction contraction of 2304 rows per tile (vs f32r's
equivalent-rate 3072), with c pre-scaled by 32 so PSUM holds 32*x.c.

The argmax is restructured so no engine outruns the PE. A classic DVE
max8 + max_index pair costs 2.44us/tile (neither op has a 2-byte fast
mode, and TensorScalarPtr is illegal on Pool), so instead the cluster
index is packed into the score and max_index is eliminated:

  ACT : u = RNE(ps + M2), M2 = 3*2^22 - the fp32 magic-number round; u
        is an exact integer + M2 (ULP 1, quantum 1/32 of x.c)
  ACT : w[232:1024] = u - M2 (Copy with float bias; exact, same binade)
  Pool: w[0:232] = u - M2 (tensor_sub with a memset M2 tile)
  Pool: v[0:352] = w + T[k],  DVE: v[352:1024] = w + T[k]
        T[k] = round(32*bias_k) + k/1024 (exact: 12 int + 10 frac bits)
  DVE : max8(v) written directly into the staging tile (slot 0 = winner;
        packed value v* = S + k/1024, |v| < 2^14)

The tail PE-transposes the staged winner columns in two halves (only the
second sits in the post-loop drain), ACT multiplies by 1024 into int32
(exact), two contiguous DMAs store [16384] i32, and the host unpacks
ids = out % 1024 after the gather. Per-tile engine budget: PE 1.93us
(bound), ACT ~1.84us, Pool ~1.78us, DVE ~1.72us, DMA ~0.6us.

Host-side layout prep (not on the device clock): fp8 hi/lo splits, the
centroid halves pre-scaled by 32 (exact power-of-2 in fp8), x
pre-transposed into DoubleRow tile layout [dlow, sc, j, n] (pairs of
128-row contraction chunks on the free axis), centroids into
[dlow, sc, j, k], T broadcast to [128, K].
"""

import sys

sys.path.insert(0, "/opt/trn_rl_repo")

import ml_dtypes
import numpy as np

N, D, K = 131072, 768, 1024
NCORES = 8
NSH = N // NCORES  # 16384 tokens per core
T = NSH // 128     # 128 token-tiles per core
SC = 3             # 256-row DoubleRow superchunks over D = 768
KHW = 512          # k half-width (one PSUM bank of fp32)
SUBQ = 232         # columns of the magic-sub done by Pool (rest on ACT)
ADDQ = 352         # columns of the pack-add done by Pool (rest on DVE)

NPF8 = ml_dtypes.float8_e4m3
MAGIC = float(3 * 2**22)  # fp32 add of this forces RNE to ULP 1

_nc_cache = []


def _build():
    from concourse import bacc, tile, mybir, masks

    f32 = mybir.dt.float32
    f8 = mybir.dt.float8e4
    i32 = mybir.dt.int32
    DR = mybir.MatmulPerfMode.DoubleRow

    nc = bacc.Bacc("TRN2", target_bir_lowering=False, debug=False)
    # xh[t, p, sc, j, n] = fp8hi(x)[t*128 + n, 256*sc + 128*j + p]
    xh_d = nc.dram_tensor("xh", [T, 128, SC, 2, 128], f8, kind="ExternalInput").ap()
    xl_d = nc.dram_tensor("xl", [T, 128, SC, 2, 128], f8, kind="ExternalInput").ap()
    # ch[p, sc, j, k] = 32*fp8hi(c)[k, 256*sc + 128*j + p]
    ch_d = nc.dram_tensor("ch", [128, SC, 2, K], f8, kind="ExternalInput").ap()
    cl_d = nc.dram_tensor("cl", [128, SC, 2, K], f8, kind="ExternalInput").ap()
    # tpack[p, k] = round(32*bias_k) + k/1024  (broadcast across partitions)
    tp_d = nc.dram_tensor("tpack", [128, K], f32, kind="ExternalInput").ap()
    out = nc.dram_tensor("out", [NSH], i32, kind="ExternalOutput").ap()

    with tile.TileContext(nc) as tc:
        with tc.tile_pool(name="const", bufs=1) as constp:
            ident = constp.tile([128, 128], f32)
            ch_s = constp.tile([128, SC, 2, K], f8)
            cl_s = constp.tile([128, SC, 2, K], f8)
            tp_s = constp.tile([128, K], f32)
            m2_s = constp.tile([128, SUBQ], f32)
            nc.vector.memset(m2_s[:], MAGIC)
            # ---- main loop over token tiles ----
            with tc.tile_pool(name="xin", bufs=3) as xinp, \
                 tc.tile_pool(name="mainps", bufs=3, space="PSUM") as psp, \
                 tc.tile_pool(name="finps", bufs=1, space="PSUM") as finp, \
                 tc.tile_pool(name="uv", bufs=3) as uvp, \
                 tc.tile_pool(name="idxcol", bufs=1) as idxp:
                # tile 0's x DMAs first, then constants in need-order with
                # few big descriptors spread over both staging queues (the
                # HWDGE feeds descriptors serially at ~630ns each)
                xh_0 = xinp.tile([128, SC, 2, 128], f8, tag="xh")
                nc.sync.dma_start(xh_0[:], xh_d[0])
                xl_0 = xinp.tile([128, SC, 2, 128], f8, tag="xl")
                nc.sync.dma_start(xl_0[:], xl_d[0])
                nc.scalar.dma_start(ch_s[:, 0], ch_d[:, 0])
                nc.scalar.dma_start(ch_s[:, 1:], ch_d[:, 1:])
                nc.gpsimd.dma_start(cl_s[:, 0], cl_d[:, 0])
                nc.gpsimd.dma_start(cl_s[:, 1], cl_d[:, 1])
                nc.gpsimd.dma_start(cl_s[:, 2], cl_d[:, 2])
                nc.scalar.dma_start(tp_s[:], tp_d[:])
                # identity built after the staging dispatches (it occupies
                # the ACT/engine sequencers and is not needed until the
                # first half-transpose at t = T//2 + 3)
                masks.make_identity(nc, ident[:])

                pcol8 = idxp.tile([128, T, 8], f32)
                for t in range(T):
                    if t == 0:
                        xh_t, xl_t = xh_0, xl_0
                    else:
                        xh_t = xinp.tile([128, SC, 2, 128], f8, tag="xh")
                        nc.sync.dma_start(xh_t[:], xh_d[t])
                        xl_t = xinp.tile([128, SC, 2, 128], f8, tag="xl")
                        nc.sync.dma_start(xl_t[:], xl_d[t])
                    ps = psp.tile([128, K], f32, tag="scps")
                    w = uvp.tile([128, K], f32, tag="w")
                    v = uvp.tile([128, K], f32, tag="v")
                    # banks interleaved per chunk and terms ordered by
                    # staging-arrival so each landing table chunk feeds 2-4
                    # matmuls, minimizing PE starvation during startup
                    # (accumulation groups are address-disjoint per bank)
                    terms = [
                        (xh_t, ch_s, 0), (xh_t, cl_s, 0), (xl_t, ch_s, 0),
                        (xh_t, ch_s, 1), (xl_t, ch_s, 1), (xh_t, cl_s, 1),
                        (xh_t, ch_s, 2), (xl_t, ch_s, 2), (xh_t, cl_s, 2),
                    ]
                    for ti, (xs, cs, sci) in enumerate(terms):
                        for kh in range(2):
                            ksl = slice(kh * KHW, (kh + 1) * KHW)
                            nc.tensor.matmul(
                                ps[:, ksl],
                                xs[:, sci, :, :],
                                cs[:, sci, :, ksl],
                                start=(ti == 0),
                                stop=(ti == len(terms) - 1),
                                perf_mode=DR,
                            )
                    # round: u = RNE(ps + MAGIC) exact integer (ULP 1);
                    # then w = u - MAGIC exact (same binade), split so every
                    # engine stays under the 18-matmul PE period: ACT takes
                    # u and 3/4 of the sub, Pool the other sub quarter.
                    u = uvp.tile([128, K], f32, tag="u")
                    nc.scalar.activation(
                        u[:], ps[:], mybir.ActivationFunctionType.Copy,
                        bias=MAGIC, scale=1.0,
                    )
                    nc.gpsimd.tensor_sub(w[:, 0:SUBQ], u[:, 0:SUBQ], m2_s[:])
                    nc.scalar.activation(
                        w[:, SUBQ:K], u[:, SUBQ:K],
                        mybir.ActivationFunctionType.Copy,
                        bias=-MAGIC, scale=1.0,
                    )
                    # v = w + (round(32b) + k/1024): exact int.frac packing
                    nc.gpsimd.tensor_add(v[:, 0:ADDQ], w[:, 0:ADDQ], tp_s[:, 0:ADDQ])
                    nc.vector.tensor_add(v[:, ADDQ:K], w[:, ADDQ:K], tp_s[:, ADDQ:K])
                    # packed argmax: winner value carries its index; top-8
                    # written straight into the staging tile (slot 0 used)
                    nc.vector.max(pcol8[:, t], v[:])

                    # transpose [token_in_tile, tile] -> [tile,
                    # token_in_tile] in two halves so only the second half
                    # sits in the post-loop drain; *1024 -> int32 makes the
                    # packed value integral (host unpacks % 1024)
                    if t == T // 2 + 3 or t == T - 1:
                        half = 0 if t < T - 1 else 1
                        hsl = slice(half * (T // 2), (half + 1) * (T // 2))
                        ftps = finp.tile([T // 2, T], f32, tag=f"ftps{half}")
                        nc.tensor.transpose(
                            ftps[:, :], pcol8[:, hsl, 0], ident[:]
                        )
                        oi = uvp.tile([T // 2, T], i32, tag=f"oi{half}")
                        nc.scalar.mul(oi[:, :], ftps[:, :], 1024.0)
                        nc.sync.dma_start(
                            out.rearrange("(t p) -> t p", p=128)[hsl], oi[:, :]
                        )

    nc.compile()
    return nc


def _get_nc():
    if not _nc_cache:
        _nc_cache.append(_build())
    return _nc_cache[0]


def _prep(x, centroids):
    x = np.ascontiguousarray(np.asarray(x), dtype=np.float32)
    c = np.ascontiguousarray(np.asarray(centroids), dtype=np.float32)

    ch8 = c.astype(NPF8)
    cl8 = (c - ch8.astype(np.float32)).astype(NPF8)
    # pre-scale by 32 (exact power-of-2 in fp8) so PSUM holds 32*x.c
    ch8 = (ch8.astype(np.float32) * 32.0).astype(NPF8)
    cl8 = (cl8.astype(np.float32) * 32.0).astype(NPF8)
    # [k, sc, j, p] -> [p, sc, j, k]
    ch_a = np.ascontiguousarray(ch8.reshape(K, SC, 2, 128).transpose(3, 1, 2, 0))
    cl_a = np.ascontiguousarray(cl8.reshape(K, SC, 2, 128).transpose(3, 1, 2, 0))

    csq = (c.astype(np.float64) ** 2).sum(-1)
    b = -0.5 * csq
    b -= b.mean()
    tpack = (np.round(32.0 * b) + np.arange(K) / 1024.0).astype(np.float32)
    tp_a = np.ascontiguousarray(np.broadcast_to(tpack.reshape(1, K), (128, K)))

    in_maps = []
    for i in range(NCORES):
        sh = x[i * NSH:(i + 1) * NSH]
        xh8 = sh.astype(NPF8)
        xl8 = (sh - xh8.astype(np.float32)).astype(NPF8)
        # [t, n, sc, j, p] -> [t, p, sc, j, n]
        xh_a = np.ascontiguousarray(
            xh8.reshape(T, 128, SC, 2, 128).transpose(0, 4, 2, 3, 1)
        )
        xl_a = np.ascontiguousarray(
            xl8.reshape(T, 128, SC, 2, 128).transpose(0, 4, 2, 3, 1)
        )
        in_maps.append(
            {"xh": xh_a, "xl": xl_a, "ch": ch_a, "cl": cl_a, "tpack": tp_a}
        )
    return in_maps


def kernel(x, centroids):
    from concourse import bass_utils

    nc = _get_nc()
    in_maps = _prep(x, centroids)
    res = bass_utils.run_bass_kernel_spmd(nc, in_maps, core_ids=list(range(NCORES)))
    packed = np.concatenate([res.results[i]["out"] for i in range(NCORES)])
    return (packed.astype(np.int64) % 1024).astype(np.int32)
